# revision 1
# baseline (speedup 1.0000x reference)
"""Trainium2 Bass kernel for nn_CvtNodeInitializer (gnn_message_passing).

Strategy (per the sharding hint: partition nodes, route edges by tail-node
owner, replicate the small projection weight):
  - Host: filter edges whose tail is a CVT node (only those contribute),
    sort by tail, and route each edge's feature rows (relation_tokens[e],
    node_tokens[e] -- the reference's "edge slot" quirk) to the core that
    owns the tail node. Nodes are partitioned contiguously: core c owns
    rows [c*25000, (c+1)*25000).
  - Device (SPMD, identical program on 8 cores): for each window of 128
    nodes, stream the window's (padded) transposed edge-feature block,
    project msg = X @ W.T with fp32 matmuls, compute per-edge logits
    with a fused multiply-reduce, exponentiate, build a one-hot(seg)*q
    matrix, and segment-reduce agg = OH.T @ msg and den = OH.T @ 1 in one
    PSUM tile. Blend agg/den + shared_cvt into the node rows via a
    predicated copy and store the window contiguously.
  - Host: concatenate the per-core output slices.
"""

import sys

sys.path.insert(0, "/opt/trn_rl_repo")

import numpy as np

N_NODES = 200000
N_EDGES = 200000
HID = 256
NCORES = 8
P = 128

_PROGRAM_CACHE: dict = {}

# matmul input dtypes: "f32" (exact, 4 cyc/row) or "f32r" (TF32-like, full
# rate at >=256 moving cols). Tunable for the accuracy/speed tradeoff.
MSG_DT = "f32"
AGG_DT = "f32"


def _build_program(S: int, W: int, repeats: int = 1):
    """Build the per-core Bass program. S = padded edge slots per window
    (<= 128), W = windows per core. Identical across cores (SPMD)."""
    import concourse.bacc as bacc
    import concourse.mybir as mybir
    import concourse.tile as tile

    f32 = mybir.dt.float32
    i32 = mybir.dt.int32
    Alu = mybir.AluOpType
    Act = mybir.ActivationFunctionType
    mdt = mybir.dt.float32r if MSG_DT == "f32r" else f32
    adt = mybir.dt.float32r if AGG_DT == "f32r" else f32

    NT = (S + P - 1) // P  # slot tiles per window
    ST = S // NT
    assert ST * NT == S and ST <= P
    G = 1
    WG = W // G

    nc = bacc.Bacc()
    xt = nc.declare_dram_parameter("xt", [WG, P, G * 4 * S], mdt,
                                   isOutput=False)
    sc = nc.declare_dram_parameter("sc", [P, W * NT], f32, isOutput=False)
    cv = nc.declare_dram_parameter("cv", [P, W], f32, isOutput=False)
    nod = nc.declare_dram_parameter("nod", [WG, P, G * HID], f32,
                                    isOutput=False)
    wch = nc.declare_dram_parameter("wch", [P, 4 * HID], mdt, isOutput=False)
    att = nc.declare_dram_parameter("att", [P, HID], f32, isOutput=False)
    shr = nc.declare_dram_parameter("shr", [P, HID], f32, isOutput=False)
    out = nc.declare_dram_parameter("out", [WG, P, G * HID], f32, isOutput=True)

    with tile.TileContext(nc) as tc:
        with (
            tc.tile_pool(name="const", bufs=1) as cpool,
            tc.tile_pool(name="x", bufs=3) as xpool,
            tc.tile_pool(name="msg", bufs=3) as mpool,
            tc.tile_pool(name="nodp", bufs=3) as npool,
            tc.tile_pool(name="small", bufs=4) as spool,
            tc.tile_pool(name="pmsg", bufs=4, space="PSUM") as pmpool,
            tc.tile_pool(name="pagg", bufs=2, space="PSUM") as papool,
            tc.tile_pool(name="pden", bufs=2, space="PSUM") as pdpool,
        ):
            # --- one-time constants ---
            wtile = cpool.tile([P, 4 * HID], mdt)
            atile = cpool.tile([P, HID], f32)
            stile = cpool.tile([P, HID], f32)
            sctile = cpool.tile([P, W * NT], f32)
            cvtile = cpool.tile([P, W], f32)
            io_i = cpool.tile([P, P], i32)
            io_f = cpool.tile([P, P], f32)
            ones = cpool.tile([P, 1], adt)
            nc.sync.dma_start(out=wtile[:], in_=wch[:])
            nc.sync.dma_start(out=atile[:], in_=att[:])
            nc.sync.dma_start(out=stile[:], in_=shr[:])
            nc.sync.dma_start(out=sctile[:], in_=sc[:])
            nc.sync.dma_start(out=cvtile[:], in_=cv[:])
            nc.gpsimd.iota(io_i[:], pattern=[[1, P]], base=0, channel_multiplier=0)
            nc.vector.tensor_copy(io_f[:], io_i[:])
            nc.gpsimd.memset(ones[:], 1.0)

            def body(w, xg, ng):
                k = w % G
                pa = papool.tile([P, HID], f32, tag="pa")
                pd = pdpool.tile([P, 16], f32, tag="pd")
                ntile = ng[:, k * HID:(k + 1) * HID]
                for t in range(NT):
                    xtile = xg[:, (k * NT + t) * 4 * ST:
                               (k * NT + t + 1) * 4 * ST]

                    # msg = X @ W_msg.T   (PSUM [ST, 256])
                    pm = pmpool.tile([P, HID], f32, tag="pm")
                    for c in range(4):
                        nc.tensor.matmul(
                            pm[:ST, :],
                            lhsT=xtile[:, c * ST:(c + 1) * ST],
                            rhs=wtile[:, c * HID:(c + 1) * HID],
                            start=(c == 0),
                            stop=(c == 3),
                        )
                    msg = mpool.tile([P, HID], adt, tag="msg")
                    nc.scalar.activation(msg[:ST, :], pm[:ST, :], Act.Copy)

                    # l = sum(msg * attn) per edge; q = exp(l)
                    scr = mpool.tile([P, HID], f32, tag="scr")
                    lq = spool.tile([P, 2], f32, tag="lq")
                    nc.vector.tensor_tensor(
                        out=scr[:ST, :], in0=msg[:ST, :], in1=atile[:ST, :],
                        op=Alu.mult,
                    )
                    nc.vector.reduce_sum(
                        out=lq[:ST, 0:1], in_=scr[:ST, :],
                        axis=mybir.AxisListType.X,
                    )
                    nc.scalar.activation(lq[:ST, 1:2], lq[:ST, 0:1], Act.Exp)

                    # one-hot(seg_local) * q   [ST edges x 128 segs]
                    oh = spool.tile([P, P], adt, tag="oh")
                    nc.vector.tensor_scalar(
                        out=oh[:ST, :],
                        in0=io_f[:ST, :],
                        scalar1=sctile[:ST, w * NT + t:w * NT + t + 1],
                        scalar2=lq[:ST, 1:2],
                        op0=Alu.is_equal,
                        op1=Alu.mult,
                    )

                    # agg += OH.T @ msg (cols 0:256), den += OH.T @ 1
                    nc.tensor.matmul(
                        pa[:, 0:HID],
                        lhsT=oh[:ST, :],
                        rhs=msg[:ST, :],
                        start=(t == 0),
                        stop=(t == NT - 1),
                    )
                    nc.tensor.matmul(
                        pd[:, 0:1],
                        lhsT=oh[:ST, :],
                        rhs=ones[:ST, :],
                        start=(t == 0),
                        stop=(t == NT - 1),
                    )

                # dsafe = den>0 ? den : 1 ; rec = 1/dsafe
                dn = spool.tile([P, 3], f32, tag="dn")
                nc.vector.tensor_copy(dn[:, 0:1], pd[:, 0:1])
                nc.vector.tensor_scalar(
                    out=dn[:, 1:2],
                    in0=dn[:, 0:1],
                    scalar1=0.0,
                    scalar2=dn[:, 0:1],
                    op0=Alu.is_le,
                    op1=Alu.add,
                )
                nc.vector.reciprocal(dn[:, 2:3], dn[:, 1:2])

                # comp = agg * rec + shared ; blend into node rows; store
                comp = mpool.tile([P, HID], f32, tag="comp")
                nc.scalar.activation(
                    comp[:], pa[:, 0:HID], Act.Copy, scale=dn[:, 2:3]
                )
                nc.vector.tensor_tensor(
                    out=comp[:], in0=comp[:], in1=stile[:], op=Alu.add
                )
                nc.vector.tensor_tensor(
                    out=comp[:], in0=comp[:], in1=ntile[:], op=Alu.subtract
                )
                nc.vector.tensor_scalar(
                    out=comp[:], in0=comp[:],
                    scalar1=cvtile[:, w:w + 1], scalar2=None, op0=Alu.mult,
                )
                nc.vector.tensor_tensor(
                    out=ntile[:], in0=ntile[:], in1=comp[:], op=Alu.add
                )

            def all_windows():
                for g in range(WG):
                    xg = xpool.tile([P, G * 4 * S], mdt, tag="xg")
                    ng = npool.tile([P, G * HID], f32, tag="ng")
                    nc.sync.dma_start(out=xg[:], in_=xt[g])
                    nc.sync.dma_start(out=ng[:], in_=nod[g])
                    for k in range(G):
                        body(g * G + k, xg, ng)
                    nc.sync.dma_start(out=out[g], in_=ng[:])

            if repeats == 1:
                all_windows()
            else:
                with tc.For_i(0, repeats, 1) as _iv:
                    all_windows()

    nc.compile()
    return nc


def _host_prep(node_tokens, relation_tokens, edge_index, node_is_cvt,
               shared_cvt, attn_vector, W_msg, n_cores=NCORES):
    """Index routing + per-core input construction. Returns (in_maps, S, W, NPC)."""
    n_nodes, hid = node_tokens.shape
    npc = n_nodes // n_cores
    assert npc * n_cores == n_nodes
    W = (npc + P - 1) // P  # windows per core
    npc_pad = W * P

    tails = np.asarray(edge_index[1], dtype=np.int64)
    cvt = np.asarray(node_is_cvt, dtype=bool)
    eids = np.nonzero(cvt[tails])[0]
    et = tails[eids]
    order = np.argsort(et, kind="stable")
    eids = eids[order]
    et = et[order]

    core = et // npc
    seg = et - core * npc
    win = (seg // P).astype(np.int64)
    gw = core * W + win
    cnt = np.bincount(gw, minlength=n_cores * W)
    starts = np.zeros(n_cores * W, dtype=np.int64)
    np.cumsum(cnt[:-1], out=starts[1:])
    rank = np.arange(len(gw)) - starts[gw]
    S = max(32, int(cnt.max()))
    NT = (S + P - 1) // P
    ST = ((S + NT - 1) // NT + 15) // 16 * 16
    S = ST * NT

    X = np.concatenate(
        [np.ascontiguousarray(relation_tokens)[eids],
         np.ascontiguousarray(node_tokens)[eids]], axis=1
    ).astype(np.float32)  # [ne, 2H]

    xt_all = np.zeros((n_cores, W, S, 2 * hid), np.float32)
    xt_all[core, win, rank] = X
    # [C,W,S,512] -> [C,W,128,NT*4*ST] with block[p, (t*4+c)*ST+j] =
    # X[slot t*ST+j, c*128+p]
    G = 1
    xt_all = np.ascontiguousarray(
        xt_all.transpose(0, 1, 3, 2)
        .reshape(n_cores, W, 4, P, NT, ST)
        .transpose(0, 1, 3, 4, 2, 5)
        .reshape(n_cores, W // G, G, P, 4 * S)
        .transpose(0, 1, 3, 2, 4)
        .reshape(n_cores, W // G, P, G * 4 * S)
    )

    segl = np.full((n_cores, W, S), -1000.0, np.float32)
    segl[core, win, rank] = (seg % P).astype(np.float32)
    segf = np.full((n_cores, W * NT, P), -1000.0, np.float32)
    segf[:, :, :ST] = segl.reshape(n_cores, W * NT, ST)
    sc_all = np.ascontiguousarray(segf.transpose(0, 2, 1))
    cvtpad = np.zeros((n_cores, npc_pad), np.float32)
    cvtpad[:, :npc] = cvt.reshape(n_cores, npc).astype(np.float32)
    cv_all = np.ascontiguousarray(
        cvtpad.reshape(n_cores, W, P).transpose(0, 2, 1)
    )

    nod_all = np.zeros((n_cores, npc_pad, hid), np.float32)
    nod_all[:, :npc] = np.asarray(node_tokens, np.float32).reshape(
        n_cores, npc, hid
    )
    nod_all = np.ascontiguousarray(
        nod_all.reshape(n_cores, W // G, G, P, hid)
        .transpose(0, 1, 3, 2, 4)
        .reshape(n_cores, W // G, P, G * hid)
    )

    Wt = np.asarray(W_msg, np.float32).T  # [2H, H]
    wch = np.ascontiguousarray(
        Wt.reshape(4, P, hid).transpose(1, 0, 2).reshape(P, 4 * hid)
    )
    att = np.ascontiguousarray(
        np.broadcast_to(np.asarray(attn_vector, np.float32), (P, hid))
    )
    shr = np.ascontiguousarray(
        np.broadcast_to(np.asarray(shared_cvt, np.float32), (P, hid))
    )

    in_maps = [
        {
            "xt": xt_all[c],
            "sc": sc_all[c],
            "cv": cv_all[c],
            "nod": nod_all[c],
            "wch": wch,
            "att": att,
            "shr": shr,
        }
        for c in range(n_cores)
    ]
    return in_maps, S, W, npc


def kernel(**inputs) -> np.ndarray:
    from concourse import bass2jax

    node_tokens = np.asarray(inputs["node_tokens"], np.float32)
    in_maps, S, W, npc = _host_prep(
        node_tokens,
        inputs["relation_tokens"],
        inputs["edge_index"],
        inputs["node_is_cvt"],
        inputs["shared_cvt"],
        inputs["attn_vector"],
        inputs["W_msg"],
    )
    key = (S, W)
    nc = _PROGRAM_CACHE.get(key)
    if nc is None:
        nc = _build_program(S, W)
        _PROGRAM_CACHE[key] = nc
    results = bass2jax.run_bass_via_pjrt(nc, in_maps, n_cores=len(in_maps))
    hid = node_tokens.shape[1]
    W_ = in_maps[0]["nod"].shape[0]  # W // G
    G_ = in_maps[0]["nod"].shape[2] // hid
    outs = []
    for r in results:
        o = (r["out"].reshape(W_, P, G_, hid)
             .transpose(0, 2, 1, 3).reshape(-1, hid)[:npc])
        outs.append(o)
    return np.concatenate(outs, axis=0)



# revision 5
# speedup vs baseline: 7.5787x; 7.5787x over previous
"""Trainium2 Bass kernel for nn_CvtNodeInitializer (gnn_message_passing).

Strategy (per the sharding hint: partition nodes, route edges by tail-node
owner, replicate the projection weight):
  - Host: filter edges whose tail is a CVT node (only those contribute),
    sort by tail, and route each edge's feature rows to the core owning
    the tail. CVT nodes are compacted and greedily packed into windows of
    <=128 nodes AND <=128 edge slots, so each window is one PSUM tile and
    one 128-slot edge tile. Non-CVT rows never touch the device: the host
    scatters the computed CVT rows into a copy of node_tokens.
  - Device (SPMD, identical program on 8 cores): per window, one fused
    matmul X^T-chunks @ [W_msg.T | a_eff] produces msg AND the per-edge
    logits (a_eff = attn @ W_msg folds the attention dot into the
    projection). exp on the scalar engine, one-hot(seg)*q on DVE, then a
    single f32r matmul segment-reduces agg = OH.T @ msg and den = OH.T @ 1.
    out_row = agg/den + shared_cvt. X and W travel as bf16 (halves HBM
    traffic; matmuls run at 1 cyc/row instead of f32's 4).
  - DMA is grouped G windows per transfer to amortize descriptor-gen cost.
"""

import sys

sys.path.insert(0, "/opt/trn_rl_repo")

import numpy as np
import ml_dtypes

N_NODES = 200000
N_EDGES = 200000
HID = 256
NCORES = 8
P = 128
G = 8           # windows per DMA group
WAUG = 272      # msg cols (256) + logit col (1) + pad

_PROGRAM_CACHE: dict = {}


def _build_program(W: int, repeats: int = 1):
    """Per-core Bass program. W = windows per core (multiple of G)."""
    import concourse.bacc as bacc
    import concourse.mybir as mybir
    import concourse.tile as tile

    f32 = mybir.dt.float32
    f32r = mybir.dt.float32r
    bf16 = mybir.dt.bfloat16
    i32 = mybir.dt.int32
    Alu = mybir.AluOpType
    Act = mybir.ActivationFunctionType

    assert W % G == 0
    WG = W // G

    nc = bacc.Bacc()
    xt = nc.declare_dram_parameter("xt", [WG, P, G * 4 * P], bf16,
                                   isOutput=False)
    sc = nc.declare_dram_parameter("sc", [P, W], f32, isOutput=False)
    wch = nc.declare_dram_parameter("wch", [P, 4 * WAUG], bf16, isOutput=False)
    shr = nc.declare_dram_parameter("shr", [P, HID], f32, isOutput=False)
    out = nc.declare_dram_parameter("out", [WG, P, G * HID], f32, isOutput=True)

    with tile.TileContext(nc) as tc:
        with (
            tc.tile_pool(name="const", bufs=1) as cpool,
            tc.tile_pool(name="x", bufs=3) as xpool,
            tc.tile_pool(name="og", bufs=2) as ogpool,
            tc.tile_pool(name="msg", bufs=3) as mpool,
            tc.tile_pool(name="small", bufs=6) as spool,
            tc.tile_pool(name="pmsg", bufs=3, space="PSUM") as pmpool,
            tc.tile_pool(name="pagg", bufs=2, space="PSUM") as papool,
            tc.tile_pool(name="pden", bufs=2, space="PSUM") as pdpool,
        ):
            # --- one-time constants ---
            wtile = cpool.tile([P, 4 * WAUG], bf16)
            stile = cpool.tile([P, HID], f32)
            sctile = cpool.tile([P, W], f32)
            io_i = cpool.tile([P, P], i32)
            io_f = cpool.tile([P, P], f32)
            ones = cpool.tile([P, 2], bf16)
            nc.sync.dma_start(out=wtile[:], in_=wch[:])
            nc.sync.dma_start(out=stile[:], in_=shr[:])
            nc.sync.dma_start(out=sctile[:], in_=sc[:])
            nc.gpsimd.iota(io_i[:], pattern=[[1, P]], base=0, channel_multiplier=0)
            nc.vector.tensor_copy(io_f[:], io_i[:])
            nc.gpsimd.memset(ones[:], 1.0)

            def stage_front(w, xg):
                """msg+logit matmul, exp, msg copy, one-hot — for window w."""
                k = w % G
                pm = pmpool.tile([P, WAUG], f32, tag="pm")
                for c in range(4):
                    nc.tensor.matmul(
                        pm[:, :],
                        lhsT=xg[:, (k * 4 + c) * P:(k * 4 + c + 1) * P],
                        rhs=wtile[:, c * WAUG:(c + 1) * WAUG],
                        start=(c == 0),
                        stop=(c == 3),
                    )
                qt = spool.tile([P, 1], f32, tag="qt")
                nc.scalar.activation(qt[:, 0:1], pm[:, 256:257], Act.Exp)
                msgb = mpool.tile([P, HID], bf16, tag="msgb")
                nc.scalar.activation(msgb[:, :], pm[:, 0:HID], Act.Copy)
                oh = spool.tile([P, P], bf16, tag="oh")
                nc.vector.tensor_scalar(
                    out=oh[:, :],
                    in0=io_f[:, :],
                    scalar1=sctile[:, w:w + 1],
                    scalar2=qt[:, 0:1],
                    op0=Alu.is_equal,
                    op1=Alu.mult,
                )
                return msgb, oh

            def stage_back(w, msgb, oh, og):
                """segment-reduce + normalize + shared add — for window w."""
                k = w % G
                pa = papool.tile([P, HID], f32, tag="pa")
                pd = pdpool.tile([P, 8], f32, tag="pd")
                nc.tensor.matmul(pa[:, :], lhsT=oh[:, :], rhs=msgb[:, :],
                                 start=True, stop=True)
                nc.tensor.matmul(pd[:, 0:2], lhsT=oh[:, :], rhs=ones[:, :],
                                 start=True, stop=True)
                dn = spool.tile([P, 2], f32, tag="dn")
                nc.vector.tensor_scalar(
                    out=dn[:, 0:1], in0=pd[:, 0:1],
                    scalar1=1e-30, scalar2=None, op0=Alu.max,
                )
                nc.vector.reciprocal(dn[:, 1:2], dn[:, 0:1])
                comp = mpool.tile([P, HID], f32, tag="comp")
                nc.scalar.activation(comp[:, :], pa[:, :], Act.Copy,
                                     scale=dn[:, 1:2])
                nc.vector.tensor_tensor(
                    out=og[:, k * HID:(k + 1) * HID],
                    in0=comp[:, :], in1=stile[:, :], op=Alu.add,
                )

            def all_windows():
                prev = None  # (w, msgb, oh, og)
                for g in range(WG):
                    xg = xpool.tile([P, G * 4 * P], bf16, tag="xg")
                    og = ogpool.tile([P, G * HID], f32, tag="og")
                    nc.sync.dma_start(out=xg[:], in_=xt[g])
                    for k in range(G):
                        w = g * G + k
                        front = stage_front(w, xg)
                        if prev is not None:
                            stage_back(*prev)
                            if prev[0] % G == G - 1:
                                pg = prev[3]
                                nc.sync.dma_start(out=out[prev[0] // G], in_=pg)
                        prev = (w, front[0], front[1], og)
                stage_back(*prev)
                nc.sync.dma_start(out=out[prev[0] // G], in_=prev[3])

            if repeats == 1:
                all_windows()
            else:
                with tc.For_i(0, repeats, 1) as _iv:
                    all_windows()

    nc.compile()
    return nc


def _host_prep(node_tokens, relation_tokens, edge_index, node_is_cvt,
               shared_cvt, attn_vector, W_msg, n_cores=NCORES):
    """Routing + per-core input construction.

    Returns (in_maps, W, scatter) where scatter = (node_ids, flat_rows)
    per core: out_full[node_ids] = dev_out[flat_rows]."""
    node_tokens = np.asarray(node_tokens, np.float32)
    relation_tokens = np.asarray(relation_tokens, np.float32)
    n_nodes, hid = node_tokens.shape

    tails = np.asarray(edge_index[1], dtype=np.int64)
    cvt = np.asarray(node_is_cvt, dtype=bool)
    cvt_nodes = np.nonzero(cvt)[0]                      # sorted CVT node ids
    ncvt = len(cvt_nodes)

    eids = np.nonzero(cvt[tails])[0]                    # contributing edges
    et = tails[eids]
    order = np.argsort(et, kind="stable")
    eids = eids[order]
    et = et[order]

    # per-CVT-node edge counts (aligned with cvt_nodes order)
    cnt_per_node = np.bincount(et, minlength=n_nodes)[cvt_nodes]
    assert cnt_per_node.max() <= P, "node with >128 edges unsupported"

    # split CVT nodes into 8 contiguous equal chunks
    bounds = [round(ncvt * c / n_cores) for c in range(n_cores + 1)]

    # greedy-pack each core's nodes into windows (<=128 nodes, <=128 edges)
    win = np.empty(ncvt, np.int64)
    seg = np.empty(ncvt, np.int64)
    estart = np.empty(ncvt, np.int64)
    Ws = []
    for c in range(n_cores):
        lo, hi = bounds[c], bounds[c + 1]
        w = 0
        nodes_in = 0
        edges_in = 0
        for i in range(lo, hi):
            k = cnt_per_node[i]
            if nodes_in == P or edges_in + k > P:
                w += 1
                nodes_in = 0
                edges_in = 0
            win[i] = w
            seg[i] = nodes_in
            estart[i] = edges_in
            nodes_in += 1
            edges_in += k
        Ws.append(w + 1 if hi > lo else 0)
    W = max(1, max(Ws))
    W = ((W + G - 1) // G) * G
    WG = W // G

    # per-edge window/slot (edges are sorted by tail; node rank via cumsum)
    node_rank_of_edge = np.searchsorted(cvt_nodes, et)   # index into cvt arrays
    first_edge_of_node = np.concatenate(
        [[0], np.cumsum(cnt_per_node)[:-1]]
    )
    rank_in_node = np.arange(len(et)) - first_edge_of_node[node_rank_of_edge]
    e_win = win[node_rank_of_edge]
    e_slot = estart[node_rank_of_edge] + rank_in_node
    core_of_node = np.searchsorted(bounds, np.arange(ncvt), side="right") - 1
    e_core = core_of_node[node_rank_of_edge]

    # edge features, routed: Xe_pad[core, w, slot] = [rel[e] | nod[e]]
    X = np.concatenate(
        [relation_tokens[eids], node_tokens[eids]], axis=1
    )                                                    # [ne, 2H] f32
    Xe = np.zeros((n_cores, W, P, 2 * hid), np.float32)
    Xe[e_core, e_win, e_slot] = X
    # xt[core, w, p, c*128+j] = Xe[core, w, j, c*128+p]; then group by G
    xt_all = (
        Xe.reshape(n_cores, W, P, 4, P)
        .transpose(0, 1, 4, 3, 2)
        .reshape(n_cores, WG, G, P, 4 * P)
        .transpose(0, 1, 3, 2, 4)
        .reshape(n_cores, WG, P, G * 4 * P)
        .astype(ml_dtypes.bfloat16)
    )

    # seg map: sc[core, slot, w] = local node id of edge in that slot
    segf = np.full((n_cores, W, P), -1000.0, np.float32)
    segf[e_core, e_win, e_slot] = seg[node_rank_of_edge].astype(np.float32)
    sc_all = np.ascontiguousarray(segf.transpose(0, 2, 1))

    # weights: wch[p, c*WAUG + h] = W_msg[h, c*128+p]; col 256 = a_eff
    a_eff = (attn_vector.astype(np.float64) @ np.asarray(W_msg, np.float64)
             ).astype(np.float32)                        # [2H]
    Wt = np.asarray(W_msg, np.float32).T                 # [2H, H]
    wch = np.zeros((P, 4 * WAUG), np.float32)
    for c in range(4):
        wch[:, c * WAUG:c * WAUG + hid] = Wt[c * P:(c + 1) * P, :]
        wch[:, c * WAUG + hid] = a_eff[c * P:(c + 1) * P]
    wch = wch.astype(ml_dtypes.bfloat16)

    shr = np.ascontiguousarray(
        np.broadcast_to(np.asarray(shared_cvt, np.float32), (P, hid))
    )

    in_maps = [
        {"xt": xt_all[c], "sc": sc_all[c], "wch": wch, "shr": shr}
        for c in range(n_cores)
    ]
    # scatter: dev_out[core] reshaped [W*P, hid] row (w*P + seg) -> node id
    scatter = []
    for c in range(n_cores):
        lo, hi = bounds[c], bounds[c + 1]
        rows = win[lo:hi] * P + seg[lo:hi]
        scatter.append((cvt_nodes[lo:hi], rows))
    return in_maps, W, scatter


def kernel(**inputs) -> np.ndarray:
    from concourse import bass2jax

    node_tokens = np.asarray(inputs["node_tokens"], np.float32)
    in_maps, W, scatter = _host_prep(
        node_tokens,
        inputs["relation_tokens"],
        inputs["edge_index"],
        inputs["node_is_cvt"],
        inputs["shared_cvt"],
        inputs["attn_vector"],
        inputs["W_msg"],
    )
    nc = _PROGRAM_CACHE.get(W)
    if nc is None:
        nc = _build_program(W)
        _PROGRAM_CACHE[W] = nc
    results = bass2jax.run_bass_via_pjrt(nc, in_maps, n_cores=len(in_maps))
    hid = node_tokens.shape[1]
    out_full = node_tokens.copy()
    for c, r in enumerate(results):
        WG = r["out"].shape[0]
        dev = (r["out"].reshape(WG, P, G, hid)
               .transpose(0, 2, 1, 3).reshape(-1, hid))  # [W*P, hid]
        node_ids, rows = scatter[c]
        out_full[node_ids] = dev[rows]
    return out_full


# revision 12
# speedup vs baseline: 7.9546x; 1.0496x over previous
"""Trainium2 Bass kernel for nn_CvtNodeInitializer (gnn_message_passing).

Strategy (per the sharding hint: partition nodes, route edges by tail-node
owner, replicate the projection weight):
  - Host: filter edges whose tail is a CVT node (only those contribute),
    sort by tail, and route each edge's feature rows to the core owning
    the tail. CVT nodes are compacted and greedily packed into windows of
    <=128 nodes AND <=128 edge slots, so each window is one PSUM tile and
    one 128-slot edge tile. Non-CVT rows never touch the device: the host
    scatters the computed CVT rows into a copy of node_tokens.
  - Device (SPMD, identical program on 8 cores): per window, one fused
    matmul X^T-chunks @ [W_msg.T | a_eff] produces msg AND the per-edge
    logits (a_eff = attn @ W_msg folds the attention dot into the
    projection). exp on the scalar engine, one-hot(seg)*q on DVE, then a
    single f32r matmul segment-reduces agg = OH.T @ msg and den = OH.T @ 1.
    out_row = agg/den + shared_cvt. X and W travel as bf16 (halves HBM
    traffic; matmuls run at 1 cyc/row instead of f32's 4).
  - DMA is grouped G windows per transfer to amortize descriptor-gen cost.
"""

import sys

sys.path.insert(0, "/opt/trn_rl_repo")

import numpy as np
import ml_dtypes

N_NODES = 200000
N_EDGES = 200000
HID = 256
NCORES = 8
P = 128
G = 8           # windows per DMA group
WAUG = 258      # msg cols (256) + logit col (1) + pad

_PROGRAM_CACHE: dict = {}


def _build_program(W: int, repeats: int = 1):
    """Per-core Bass program. W = windows per core (multiple of G)."""
    import concourse.bacc as bacc
    import concourse.mybir as mybir
    import concourse.tile as tile

    f32 = mybir.dt.float32
    f32r = mybir.dt.float32r
    bf16 = mybir.dt.bfloat16
    i32 = mybir.dt.int32
    Alu = mybir.AluOpType
    Act = mybir.ActivationFunctionType

    assert W % G == 0
    WG = W // G

    nc = bacc.Bacc()
    xt = nc.declare_dram_parameter("xt", [WG, P, G * 4 * P], bf16,
                                   isOutput=False)
    sc = nc.declare_dram_parameter("sc", [P, W], f32, isOutput=False)
    wch = nc.declare_dram_parameter("wch", [P, 4 * WAUG], bf16, isOutput=False)
    out = nc.declare_dram_parameter("out", [WG, P, G * HID], bf16,
                                    isOutput=True)

    with tile.TileContext(nc) as tc:
        with (
            tc.tile_pool(name="const", bufs=1) as cpool,
            tc.tile_pool(name="x", bufs=3) as xpool,
            tc.tile_pool(name="og", bufs=2) as ogpool,
            tc.tile_pool(name="msg", bufs=3) as mpool,
            tc.tile_pool(name="small", bufs=6) as spool,
            tc.tile_pool(name="pmsg", bufs=3, space="PSUM") as pmpool,
            tc.tile_pool(name="pagg", bufs=2, space="PSUM") as papool,
            tc.tile_pool(name="pden", bufs=2, space="PSUM") as pdpool,
        ):
            # --- one-time constants ---
            wtile = cpool.tile([P, 4 * WAUG], bf16)
            sctile = cpool.tile([P, W], f32)
            io_i = cpool.tile([P, P], i32)
            io_f = cpool.tile([P, P], f32)
            ones = cpool.tile([P, 2], bf16)
            nc.sync.dma_start(out=wtile[:], in_=wch[:])
            nc.sync.dma_start(out=sctile[:], in_=sc[:])
            nc.gpsimd.iota(io_i[:], pattern=[[1, P]], base=0, channel_multiplier=0)
            nc.vector.tensor_copy(io_f[:], io_i[:])
            nc.gpsimd.memset(ones[:], 1.0)

            def stage_front(w, xg):
                """msg+logit matmul, exp, msg copy, one-hot — for window w."""
                k = w % G
                pm = pmpool.tile([P, WAUG], f32, tag="pm")
                for c in range(4):
                    nc.tensor.matmul(
                        pm[:, :],
                        lhsT=xg[:, (k * 4 + c) * P:(k * 4 + c + 1) * P],
                        rhs=wtile[:, c * WAUG:(c + 1) * WAUG],
                        start=(c == 0),
                        stop=(c == 3),
                    )
                qt = spool.tile([P, 1], f32, tag="qt")
                nc.scalar.activation(qt[:, 0:1], pm[:, 256:257], Act.Exp)
                msgb = mpool.tile([P, HID], bf16, tag="msgb")
                nc.scalar.activation(msgb[:, :], pm[:, 0:HID], Act.Copy)
                oh = spool.tile([P, P], bf16, tag="oh")
                nc.vector.tensor_scalar(
                    out=oh[:, :],
                    in0=io_f[:, :],
                    scalar1=sctile[:, w:w + 1],
                    scalar2=qt[:, 0:1],
                    op0=Alu.is_equal,
                    op1=Alu.mult,
                )
                return msgb, oh

            def stage_back(w, msgb, oh, og):
                """segment-reduce + normalize + shared add — for window w."""
                k = w % G
                pa = papool.tile([P, HID], f32, tag="pa")
                pd = pdpool.tile([P, 8], f32, tag="pd")
                nc.tensor.matmul(pa[:, :], lhsT=oh[:, :], rhs=msgb[:, :],
                                 start=True, stop=True)
                nc.tensor.matmul(pd[:, 0:2], lhsT=oh[:, :], rhs=ones[:, :],
                                 start=True, stop=True)
                dn = spool.tile([P, 2], f32, tag="dn")
                nc.vector.tensor_scalar(
                    out=dn[:, 0:1], in0=pd[:, 0:1],
                    scalar1=1e-30, scalar2=None, op0=Alu.max,
                )
                nc.vector.reciprocal(dn[:, 1:2], dn[:, 0:1])
                nc.scalar.activation(og[:, k * HID:(k + 1) * HID], pa[:, :],
                                     Act.Copy, scale=dn[:, 1:2])

            def all_windows():
                prev = None  # (w, msgb, oh, og)
                for g in range(WG):
                    xg = xpool.tile([P, G * 4 * P], bf16, tag="xg")
                    og = ogpool.tile([P, G * HID], bf16, tag="og")
                    nc.sync.dma_start(out=xg[:], in_=xt[g])
                    for k in range(G):
                        w = g * G + k
                        front = stage_front(w, xg)
                        if prev is not None:
                            stage_back(*prev)
                            if prev[0] % G == G - 1:
                                pg = prev[3]
                                nc.sync.dma_start(out=out[prev[0] // G], in_=pg)
                        prev = (w, front[0], front[1], og)
                stage_back(*prev)
                nc.sync.dma_start(out=out[prev[0] // G], in_=prev[3])

            if repeats == 1:
                all_windows()
            else:
                with tc.For_i(0, repeats, 1) as _iv:
                    all_windows()

    nc.compile()
    return nc


def _host_prep(node_tokens, relation_tokens, edge_index, node_is_cvt,
               shared_cvt, attn_vector, W_msg, n_cores=NCORES):
    """Routing + per-core input construction.

    Returns (in_maps, W, scatter) where scatter = (node_ids, flat_rows)
    per core: out_full[node_ids] = dev_out[flat_rows]."""
    node_tokens = np.asarray(node_tokens, np.float32)
    relation_tokens = np.asarray(relation_tokens, np.float32)
    n_nodes, hid = node_tokens.shape

    tails = np.asarray(edge_index[1], dtype=np.int64)
    cvt = np.asarray(node_is_cvt, dtype=bool)
    cvt_nodes = np.nonzero(cvt)[0]                      # sorted CVT node ids
    ncvt = len(cvt_nodes)

    eids = np.nonzero(cvt[tails])[0]                    # contributing edges
    et = tails[eids]
    order = np.argsort(et, kind="stable")
    eids = eids[order]
    et = et[order]

    # per-CVT-node edge counts (aligned with cvt_nodes order)
    cnt_per_node = np.bincount(et, minlength=n_nodes)[cvt_nodes]
    assert cnt_per_node.max() <= P, "node with >128 edges unsupported"

    # split CVT nodes into 8 contiguous equal chunks
    bounds = [round(ncvt * c / n_cores) for c in range(n_cores + 1)]

    # greedy-pack each core's nodes into windows (<=128 nodes, <=128 edges)
    win = np.empty(ncvt, np.int64)
    seg = np.empty(ncvt, np.int64)
    estart = np.empty(ncvt, np.int64)
    Ws = []
    for c in range(n_cores):
        lo, hi = bounds[c], bounds[c + 1]
        w = 0
        nodes_in = 0
        edges_in = 0
        for i in range(lo, hi):
            k = cnt_per_node[i]
            if nodes_in == P or edges_in + k > P:
                w += 1
                nodes_in = 0
                edges_in = 0
            win[i] = w
            seg[i] = nodes_in
            estart[i] = edges_in
            nodes_in += 1
            edges_in += k
        Ws.append(w + 1 if hi > lo else 0)
    W = max(1, max(Ws))
    W = ((W + G - 1) // G) * G
    WG = W // G

    # per-edge window/slot (edges are sorted by tail; node rank via cumsum)
    node_rank_of_edge = np.searchsorted(cvt_nodes, et)   # index into cvt arrays
    first_edge_of_node = np.concatenate(
        [[0], np.cumsum(cnt_per_node)[:-1]]
    )
    rank_in_node = np.arange(len(et)) - first_edge_of_node[node_rank_of_edge]
    e_win = win[node_rank_of_edge]
    e_slot = estart[node_rank_of_edge] + rank_in_node
    core_of_node = np.searchsorted(bounds, np.arange(ncvt), side="right") - 1
    e_core = core_of_node[node_rank_of_edge]

    # edge features, routed: Xe_pad[core, w, slot] = [rel[e] | nod[e]]
    X = np.concatenate(
        [relation_tokens[eids], node_tokens[eids]], axis=1
    )                                                    # [ne, 2H] f32
    Xe = np.zeros((n_cores, W, P, 2 * hid), np.float32)
    Xe[e_core, e_win, e_slot] = X
    # xt[core, w, p, c*128+j] = Xe[core, w, j, c*128+p]; then group by G
    xt_all = (
        Xe.reshape(n_cores, W, P, 4, P)
        .transpose(0, 1, 4, 3, 2)
        .reshape(n_cores, WG, G, P, 4 * P)
        .transpose(0, 1, 3, 2, 4)
        .reshape(n_cores, WG, P, G * 4 * P)
        .astype(ml_dtypes.bfloat16)
    )

    # seg map: sc[core, slot, w] = local node id of edge in that slot
    segf = np.full((n_cores, W, P), -1000.0, np.float32)
    segf[e_core, e_win, e_slot] = seg[node_rank_of_edge].astype(np.float32)
    sc_all = np.ascontiguousarray(segf.transpose(0, 2, 1))

    # weights: wch[p, c*WAUG + h] = W_msg[h, c*128+p]; col 256 = a_eff
    a_eff = (attn_vector.astype(np.float64) @ np.asarray(W_msg, np.float64)
             ).astype(np.float32)                        # [2H]
    Wt = np.asarray(W_msg, np.float32).T                 # [2H, H]
    wch = np.zeros((P, 4 * WAUG), np.float32)
    for c in range(4):
        wch[:, c * WAUG:c * WAUG + hid] = Wt[c * P:(c + 1) * P, :]
        wch[:, c * WAUG + hid] = a_eff[c * P:(c + 1) * P]
    wch = wch.astype(ml_dtypes.bfloat16)

    in_maps = [
        {"xt": xt_all[c], "sc": sc_all[c], "wch": wch}
        for c in range(n_cores)
    ]
    # scatter: dev_out[core] reshaped [W*P, hid] row (w*P + seg) -> node id
    scatter = []
    for c in range(n_cores):
        lo, hi = bounds[c], bounds[c + 1]
        rows = win[lo:hi] * P + seg[lo:hi]
        scatter.append((cvt_nodes[lo:hi], rows))
    return in_maps, W, scatter


def kernel(**inputs) -> np.ndarray:
    from concourse import bass2jax

    node_tokens = np.asarray(inputs["node_tokens"], np.float32)
    in_maps, W, scatter = _host_prep(
        node_tokens,
        inputs["relation_tokens"],
        inputs["edge_index"],
        inputs["node_is_cvt"],
        inputs["shared_cvt"],
        inputs["attn_vector"],
        inputs["W_msg"],
    )
    nc = _PROGRAM_CACHE.get(W)
    if nc is None:
        nc = _build_program(W)
        _PROGRAM_CACHE[W] = nc
    results = bass2jax.run_bass_via_pjrt(nc, in_maps, n_cores=len(in_maps))
    hid = node_tokens.shape[1]
    shared = np.asarray(inputs["shared_cvt"], np.float32)
    out_full = node_tokens.copy()
    for c, r in enumerate(results):
        WG = r["out"].shape[0]
        dev = (np.asarray(r["out"]).astype(np.float32)
               .reshape(WG, P, G, hid)
               .transpose(0, 2, 1, 3).reshape(-1, hid))  # [W*P, hid]
        node_ids, rows = scatter[c]
        out_full[node_ids] = dev[rows] + shared
    return out_full
